# revision 41
# baseline (speedup 1.0000x reference)
"""MoE routing kernel (2 experts, D=128 -> H=512 -> O=2) for 8 Trainium2 cores.

Strategy: host-side routing + expert-sorted pure data parallelism.

The routing decision (argmin over 2 prototypes == a 1-D threshold test
q = x.(p1-p0) vs (|p1|^2-|p0|^2)/2) is computed on the host, and samples are
re-ordered so every 512-sample device block is single-expert. This halves the
matmul work vs. computing both experts and selecting. The host also feeds x
pre-transposed ([D, n] layout, bf16), so the device does no transposes and no
routing.

Device schedule, in groups of up to 4 blocks (2048 samples):
  - per-block DMAs bring xT [128d, 512b] (bf16)
  - layer 1 j-major: per hidden k-tile j, G matmuls (w1_{e,j} stationary);
    relu+bias runs on pairs of PSUM banks [128, 1024], greedily load-balanced
    between ACT and DVE (the only engines that can read PSUM)
  - layer 2 col-tiled: per j, G concurrent matmuls (tile_position=(0,32g),
    M=2) accumulate all G blocks' outputs into ONE psum bank at partition
    offsets 32g; a single [128, 512] copy evacuates the whole group. Layer 2
    of group k runs as one contiguous chunk in the middle of group k+1's
    layer 1 (software pipelining), so the PE never waits on a fresh relu.
  - one [128, 512] DMA per group writes the padded outputs (b2 added on host)

Per-expert sample counts are rounded DOWN to full 4096-sample device slabs;
the remainder (<4096 per expert) is computed on the host in fp32 numpy, so
device blocks are always completely full. Programs are compiled per (m0, m1)
and cached.
"""

import numpy as np
import ml_dtypes

import concourse.bacc as bacc
import concourse.bass as bass
import concourse.mybir as mybir
import concourse.tile as tile
from concourse.bass_utils import run_bass_kernel_spmd

F32 = mybir.dt.float32
BF16 = mybir.dt.bfloat16
NP_BF16 = ml_dtypes.bfloat16

N_CORES = 8
D = 128
H = 512
E = 2
O = 2
NJ = H // 128         # 4 hidden k-tiles of 128 per expert
BLK = 512             # samples per block (one PSUM bank of fp32)
G = 4                 # blocks per group


def _build_program(m0: int, m1: int):
    """Per-core program: m0 expert-0 blocks then m1 expert-1 blocks."""
    nblk = m0 + m1
    n_slots = nblk * BLK

    nc = bacc.Bacc(
        "TRN2",
        target_bir_lowering=False,
        debug=False,
        enable_asserts=False,
        num_devices=1,
    )

    xtd = nc.dram_tensor("xtd", [D, n_slots], BF16, kind="ExternalInput").ap()
    w1p = nc.dram_tensor("w1p", [D, E * H], BF16, kind="ExternalInput").ap()
    w2p = nc.dram_tensor("w2p", [128, E * NJ * O], BF16, kind="ExternalInput").ap()
    b1p = nc.dram_tensor("b1p", [128, E * NJ], F32, kind="ExternalInput").ap()
    ngrp = len(list(_groups(m0, m1)))
    # group outputs, padded: rows 32g..32g+1 of group gi hold block (gi,g)'s
    # [O, BLK]; host slices the useful rows out
    out = nc.dram_tensor("out", [ngrp, 128, BLK], F32, kind="ExternalOutput").ap()

    with tile.TileContext(nc) as tc:
        _body(tc, m0, m1, xtd, w1p, w2p, b1p, out)

    nc.compile()
    return nc


def _groups(m0, m1):
    """Yield (start_block, n_blocks_in_group, expert), sizes balanced."""
    for base, m, e in ((0, m0, 0), (m0, m1, 1)):
        if m <= 0:
            continue
        k = -(-m // G)  # number of groups for this expert region
        b = 0
        for i in range(k):
            g = m * (i + 1) // k - m * i // k
            yield base + b, g, e
            b += g


def _body(tc, m0, m1, xtd, w1p, w2p, b1p, out):
    nc = tc.nc
    Relu = mybir.ActivationFunctionType.Relu
    Alu = mybir.AluOpType

    with (
        tc.tile_pool(name="consts", bufs=1) as cpool,
        tc.tile_pool(name="xt", bufs=10) as xt_pool,
        tc.tile_pool(name="h", bufs=3) as h_pool,
        tc.tile_pool(name="osb", bufs=3) as o_pool,
        tc.tile_pool(name="zp", bufs=3, space="PSUM") as zp_pool,
        tc.tile_pool(name="op", bufs=2, space="PSUM") as op_pool,
    ):
        xv = xtd.rearrange("p (n b) -> p n b", b=BLK)
        glist = list(_groups(m0, m1))
        e_first = glist[0][2]

        # --- bootstrap: HWDGE executes FIFO per ring, so issue exactly what
        # the first matmuls need before the bulk constants ---
        w1_sb = cpool.tile([D, E, H], BF16)
        w1v = w1p.rearrange("p (e h) -> p e h", e=E)
        nc.sync.dma_start(
            w1_sb[:, e_first, 0:128], w1v[:, e_first, 0:128]
        )
        b00, g0, _ = glist[0]
        xts0 = []
        for gi in range(g0):
            xt = xt_pool.tile([D, BLK], BF16)
            nc.sync.dma_start(xt[:], xv[:, b00 + gi, :])
            xts0.append(xt[:])
        for j in range(1, NJ):
            nc.sync.dma_start(
                w1_sb[:, e_first, j * 128 : (j + 1) * 128],
                w1v[:, e_first, j * 128 : (j + 1) * 128],
            )
        b1_sb = cpool.tile([128, E, NJ], F32)
        nc.sync.dma_start(b1_sb[:], b1p.rearrange("p (e j) -> p e j", e=E))
        w2_sb = cpool.tile([128, E, NJ, O], BF16)
        nc.sync.dma_start(w2_sb[:], w2p.rearrange("p (e j o) -> p e j o", e=E, j=NJ))
        e_other = 1 - e_first
        if (m0 if e_other == 0 else m1) > 0:
            nc.sync.dma_start(w1_sb[:, e_other, :], w1v[:, e_other, :])

        # greedy ACT/DVE load balancing (GPSIMD cannot read PSUM).
        # projected per-op ns: ACT ~0.833/col + 260 fixed, DVE ~1.042/col + 190
        load = [0.0, 0.0]  # [ACT, DVE]

        def psum_op(ncols, make_act, make_dve):
            cost = (0.833 * ncols + 260, 1.042 * ncols + 190)
            eng = 0 if load[0] + cost[0] <= load[1] + cost[1] else 1
            load[eng] += cost[eng]
            (make_act if eng == 0 else make_dve)()

        def relu_op(dst, src, bias, ncols):
            psum_op(
                ncols,
                lambda: nc.scalar.activation(dst, src, Relu, bias=bias, scale=1.0),
                lambda: nc.vector.tensor_scalar(
                    out=dst, in0=src, scalar1=bias, scalar2=0.0,
                    op0=Alu.add, op1=Alu.max,
                ),
            )

        def evac_op(dst, src, ncols):
            psum_op(
                ncols,
                lambda: nc.scalar.copy(dst, src),
                lambda: nc.vector.tensor_copy(dst, src),
            )

        def l2_step(pend, j):
            # one layer-2 j-step for the *previous* group: g concurrent
            # col-tiled matmuls (tile_position=(0,32g)) into one psum bank.
            h, e, g, gidx, op_ps = pend
            for gk in range(g):
                nc.tensor.matmul(
                    op_ps[32 * gk : 32 * gk + O, :],
                    lhsT=w2_sb[:, e, j, :],
                    rhs=h[:, j, gk, :],
                    start=(j == 0),
                    stop=(j == NJ - 1),
                    tile_position=(0, 32 * gk),
                )

        def l2_finish(pend, final=False):
            _, _, g, gidx, op_ps = pend
            osb = o_pool.tile([128, BLK], F32)
            evac_op(osb[:], op_ps[:], BLK)
            # issue the store on the (idle) GpSimd SWDGE ring: an out-DMA
            # waiting for its evac at the head of the SP HWDGE FIFO would
            # block all later xt-prefetch DMAs queued behind it
            if final:
                # per-block slices: the kernel's last transfer is 4KB, not
                # 256KB, so the closing sem settles sooner
                for gk in range(g):
                    nc.gpsimd.dma_start(
                        out[gidx, 32 * gk : 32 * gk + O, :],
                        osb[32 * gk : 32 * gk + O, :],
                    )
            else:
                nc.gpsimd.dma_start(out[gidx], osb[:])

        pending = None
        for gidx, (b0, g, e) in enumerate(glist):
            last = gidx == len(glist) - 1
            # group 0: per-block DMAs for the fastest possible start; later
            # groups: one DMA per group to keep the SP issue queue light
            if gidx == 0:
                xts = xts0
            else:
                xtg = xt_pool.tile([D, G, BLK], BF16, tag="xtg")
                nc.sync.dma_start(xtg[:, :g, :], xv[:, b0 : b0 + g, :])
                xts = [xtg[:, gi, :] for gi in range(g)]

            h = h_pool.tile([128, NJ, g, BLK], BF16)
            op_ps = op_pool.tile([128, BLK], F32)
            selfp = (h, e, g, gidx, op_ps)
            for j in range(NJ):
                pairs = [(p0, min(p0 + 2, g)) for p0 in range(0, g, 2)]
                for p0, p1 in pairs:
                    zp = zp_pool.tile([128, 2, BLK], F32, tag="zp")
                    for gi in range(p0, p1):
                        nc.tensor.matmul(
                            zp[:, gi - p0, :],
                            lhsT=w1_sb[:, e, j * 128 : (j + 1) * 128],
                            rhs=xts[gi],
                            start=True,
                            stop=True,
                        )
                    relu_op(
                        h[:, j, p0:p1, :],
                        zp[:, : p1 - p0, :],
                        b1_sb[:, e, j : j + 1],
                        (p1 - p0) * BLK,
                    )
                # previous group's layer 2 as one contiguous chunk after j=1:
                # a single L1<->L2 transition (~600ns) instead of four, placed
                # mid-group where relu backpressure on the zp ring peaks.
                # The evac is deferred to j=3: emitted right after the chunk
                # it would block the rail FIFO head until all 4 j-steps finish
                if pending is not None and j == 1:
                    for jp in range(NJ):
                        l2_step(pending, jp)
                if pending is not None and j == 3:
                    l2_finish(pending)
                # final group: weave its own layer 2, lagged one j behind the
                # relus, so the kernel tail is just one j-step + evac
                if last and j >= 1:
                    l2_step(selfp, j - 1)
            if last:
                l2_step(selfp, NJ - 1)
                l2_finish(selfp, final=True)
            else:
                pending = selfp


def _pack_consts(w1, b1, w2):
    w1 = np.asarray(w1, np.float32)
    b1 = np.asarray(b1, np.float32)
    w2 = np.asarray(w2, np.float32)

    # w1p[d, e, h] = w1[e, h, d]
    w1p = np.ascontiguousarray(np.transpose(w1, (2, 0, 1)).reshape(D, E * H))
    # w2p[p, e, j, o] = w2[e, o, j*128+p]
    w2p = np.ascontiguousarray(
        np.transpose(w2.reshape(E, O, NJ, 128), (3, 0, 2, 1)).reshape(128, E * NJ * O)
    )
    # b1p[p, e, j] = b1[e, j*128+p]
    b1p = np.ascontiguousarray(
        np.transpose(b1.reshape(E, NJ, 128), (2, 0, 1)).reshape(128, E * NJ)
    )
    return dict(
        w1p=w1p.astype(NP_BF16),
        w2p=w2p.astype(NP_BF16),
        b1p=b1p,
    )


_PROG_CACHE = {}


def _get_program(m0, m1):
    key = (m0, m1)
    if key not in _PROG_CACHE:
        _PROG_CACHE[key] = _build_program(m0, m1)
    return _PROG_CACHE[key]


def kernel(x, w1, b1, w2, b2, prototypes, _trace=False):
    x = np.ascontiguousarray(np.asarray(x, np.float32))
    btot = x.shape[0]

    # host routing: expert = argmin_e |x - p_e|^2  ==  1 if q > thr else 0
    p = np.asarray(prototypes, np.float64)
    rvec = p[1] - p[0]
    thr = (p[1] @ p[1] - p[0] @ p[0]) / 2.0
    q = x.astype(np.float64) @ rvec
    is1 = q > thr
    sel0 = np.flatnonzero(~is1)
    sel1 = np.flatnonzero(is1)
    n0, n1 = sel0.size, sel1.size

    # per-core expert block counts. Prefer rounding DOWN and absorbing the
    # small remainder on the host (full fp32 numpy MLP) — this avoids a
    # nearly-empty ragged device block per expert.
    HOST_ABSORB = 4096  # max leftover samples (per expert) computed on host

    def split_counts(n):
        cap = N_CORES * BLK
        m_floor = n // cap
        rem = n - m_floor * cap
        if m_floor >= 1 and 0 < rem <= HOST_ABSORB:
            return m_floor, rem
        return -(-n // cap), 0

    m0, rem0 = split_counts(n0)
    m1, rem1 = split_counts(n1)
    host0 = sel0[n0 - rem0 :]
    host1 = sel1[n1 - rem1 :]
    sel0 = sel0[: n0 - rem0]
    sel1 = sel1[: n1 - rem1]
    n0 -= rem0
    n1 -= rem1
    n_slots = (m0 + m1) * BLK

    nc = _get_program(m0, m1)
    consts = _pack_consts(w1, b1, w2)
    b2 = np.asarray(b2, np.float32)

    x_bf = x.astype(NP_BF16)
    # split sample lists across cores (sizes differ by at most 1)
    bounds0 = [n0 * c // N_CORES for c in range(N_CORES + 1)]
    bounds1 = [n1 * c // N_CORES for c in range(N_CORES + 1)]

    in_maps = []
    core_sel = []
    for c in range(N_CORES):
        s0 = sel0[bounds0[c] : bounds0[c + 1]]
        s1 = sel1[bounds1[c] : bounds1[c + 1]]
        xs = np.zeros((n_slots, D), NP_BF16)
        xs[: s0.size] = x_bf[s0]
        xs[m0 * BLK : m0 * BLK + s1.size] = x_bf[s1]
        m = dict(consts)
        m["xtd"] = np.ascontiguousarray(xs.T)
        in_maps.append(m)
        core_sel.append((s0, s1))

    res = run_bass_kernel_spmd(
        nc, in_maps, core_ids=list(range(N_CORES)), trace=_trace
    )

    # reassemble: out[gidx, 32g:32g+2, :] holds block (b0+g)'s [O, BLK]
    full = np.empty((btot, O), np.float32)
    for c in range(N_CORES):
        s0, s1 = core_sel[c]
        ot = res.results[c]["out"]  # [ngrp, 128, BLK]
        flat = np.empty((n_slots, O), np.float32)
        for gidx, (b0, g, e) in enumerate(_groups(m0, m1)):
            for gi in range(g):
                blk = b0 + gi
                flat[blk * BLK : (blk + 1) * BLK] = ot[
                    gidx, 32 * gi : 32 * gi + O, :
                ].T
        full[s0] = flat[: s0.size] + b2[0]
        full[s1] = flat[m0 * BLK : m0 * BLK + s1.size] + b2[1]

    # host-absorbed remainder samples (exact fp32)
    w1f = np.asarray(w1, np.float32)
    b1f = np.asarray(b1, np.float32)
    w2f = np.asarray(w2, np.float32)
    for e, idx in ((0, host0), (1, host1)):
        if idx.size:
            hh = np.maximum(x[idx] @ w1f[e].T + b1f[e], 0.0)
            full[idx] = hh @ w2f[e].T + b2[e]
    if _trace:
        return full, res
    return full
